# revision 1
# baseline (speedup 1.0000x reference)
"""DiGCN (2-layer GCNConv + parallel Linear + BatchNorm1d + ReLU) on 8 trn2 NeuronCores.

Strategy (matches the problem's sharding hint):
  - Shard nodes contiguously across 8 cores (12500 nodes/core), replicate the
    small [D,D] weights, partition edges by destination-node owner.
  - Per layer: each core computes hg = h_local @ gcn_w.T (bf16), AllGathers hg
    so every core holds the full [N,D] table; gathers its edges' source rows
    via the GPSIMD dma_gather extended instruction; scatter-adds into its local
    destination nodes via one-hot matmuls on the TensorEngine (edges sorted by
    dest tile; a host-built [128e x 128d] one-hot*norm bf16 matrix per 128-edge
    chunk turns segment-sum into PSUM accumulation). BN stats via AllReduce.
  - dma_gather indices are int16, so the [N,D] table is addressed through
    range buckets of 32767 rows; slots are laid out bucket-major per batch so
    each (batch, bucket) is one packed gather call.
  - h is kept transposed in SBUF ([128 feat, nodes], f32) so BN reduction is a
    free-dim reduce and BN+ReLU fuse into one ScalarE activation pass.

kernel(**inputs) takes FULL inputs, returns the FULL [N,D] float32 output.
"""

import math
import os
import sys

import numpy as np

for _p in ("/opt/trn_rl_repo", "/root/.axon_site/_ro/trn_rl_repo"):
    if os.path.isdir(_p) and _p not in sys.path:
        sys.path.insert(0, _p)

# ---------------------------------------------------------------- configuration
N_GLOBAL = 100000
E_GLOBAL = 500000
D = 128
DEPTH = 2
EPS = 1e-5
NCORES = 8
IDXMAX = 32767  # dma_gather int16 index limit (rows per range bucket)

LAST_RUNNER = None  # (run_once, fetch) of the most recent kernel() call


class _Cfg:
    def __init__(self, n_global, ncores, depth=DEPTH, eps=EPS, idxmax=IDXMAX, tb=8):
        assert n_global % ncores == 0
        self.n_global = n_global
        self.ncores = ncores
        self.depth = depth
        self.eps = eps
        self.idxmax = idxmax
        self.tb = tb                                # dest tiles per gather batch
        self.np_local = n_global // ncores          # real nodes per core
        self.nt = math.ceil(self.np_local / 128)    # dest tiles per core
        self.npad = self.nt * 128                   # padded nodes per core
        self.nb = math.ceil(n_global / idxmax)      # gather range buckets
        self.bases = [b * idxmax for b in range(self.nb)]


def _layout(cfg, K):
    """Chunk/call enumeration shared by host prep and the bass builder.

    Returns (chunk_meta, call_meta, cidx, batches):
      chunk_meta[c] = (bucket, tile, j)
      call_meta = list of (batch_idx, bucket, c0, nchunks_in_call)
      cidx[(t, b, j)] = global chunk index
      batches = list of (t0, t1, bc0, bc1)  (tile range, chunk range)
    """
    chunk_meta, call_meta, cidx, batches = [], [], {}, []
    nbatch = math.ceil(cfg.nt / cfg.tb)
    for bi in range(nbatch):
        t0, t1 = bi * cfg.tb, min(cfg.nt, bi * cfg.tb + cfg.tb)
        bc0 = len(chunk_meta)
        for b in range(cfg.nb):
            if K[b] == 0:
                continue
            c0 = len(chunk_meta)
            for t in range(t0, t1):
                for j in range(K[b]):
                    cidx[(t, b, j)] = len(chunk_meta)
                    chunk_meta.append((b, t, j))
            call_meta.append((bi, b, c0, len(chunk_meta) - c0))
        batches.append((t0, t1, bc0, len(chunk_meta)))
    return chunk_meta, call_meta, cidx, batches


# ---------------------------------------------------------------- host-side prep
def _prep_graph(cfg, edge_index, edge_weight):
    """Partition edges by destination owner/tile/src-bucket; build int16 gather
    indices and the per-chunk one-hot*norm matrices (bf16)."""
    import ml_dtypes

    row = np.asarray(edge_index[0], dtype=np.int64)
    col = np.asarray(edge_index[1], dtype=np.int64)
    w = np.asarray(edge_weight, dtype=np.float32)
    n, nb, nt, npl = cfg.n_global, cfg.nb, cfg.nt, cfg.np_local

    deg = np.bincount(col, weights=w.astype(np.float64), minlength=n).astype(np.float32)
    dinv = np.where(deg > 0, 1.0 / np.sqrt(np.where(deg > 0, deg, 1.0)), 0.0).astype(
        np.float32
    )
    norm = (dinv[row] * w * dinv[col]).astype(np.float32)

    core = col // npl
    lc = col % npl
    tile = lc // 128
    d_in_tile = lc % 128
    bucket = np.minimum(row // cfg.idxmax, nb - 1)

    key = (core * nt + tile) * nb + bucket
    order = np.argsort(key, kind="stable")
    key_s = key[order]
    counts = np.bincount(key, minlength=cfg.ncores * nt * nb)
    starts = np.concatenate([[0], np.cumsum(counts)[:-1]])
    rank = np.arange(row.shape[0], dtype=np.int64) - starts[key_s]

    cnt3 = counts.reshape(cfg.ncores, nt, nb)
    K = [int(math.ceil(cnt3[:, :, b].max() / 128)) for b in range(nb)]

    chunk_meta, call_meta, cidx, batches = _layout(cfg, K)
    nchunks = len(chunk_meta)

    # per-edge slot: chunk c = cidx[(tile, bucket, rank//128)], partition rank%128
    cidx_arr = np.full((nt, nb, max(K)), -1, dtype=np.int64)
    for (t, b, j), c in cidx.items():
        cidx_arr[t, b, j] = c
    t_s = (key_s // nb) % nt
    b_s = key_s % nb
    core_s = key_s // (nb * nt)
    j_s = rank // 128
    p_s = rank % 128
    c_s = cidx_arr[t_s, b_s, j_s]
    assert (c_s >= 0).all()

    base_arr = np.asarray(cfg.bases, dtype=np.int64)
    val_s = (row[order] - base_arr[b_s]).astype(np.int16)
    assert (val_s >= 0).all()

    # int16 gather indices: element i of a call lives at [i%16, chunk*8 + p//16]
    idx_all = np.zeros((cfg.ncores, 128, nchunks * 8), dtype=np.int16)
    idx_all[core_s, p_s % 16, c_s * 8 + p_s // 16] = val_s
    idx_all[:, 16:32, :] = idx_all[:, 0:16, :]  # replica for the 2nd Q7 cpu

    mt_all = np.zeros((cfg.ncores, 128, nchunks * 128), dtype=ml_dtypes.bfloat16)
    mt_all[core_s, p_s, c_s * 128 + d_in_tile[order]] = norm[order].astype(
        ml_dtypes.bfloat16
    )
    return K, idx_all, mt_all


def _prep_inputs(cfg, K, idx_all, mt_all, x, lin_w, gcn_w, gamma, beta):
    x = np.asarray(x, dtype=np.float32)
    npl, npad = cfg.np_local, cfg.npad
    wlin = np.concatenate([lin_w[i].T for i in range(cfg.depth)], axis=1).astype(
        np.float32
    )  # [D, depth*D], column block i = lin_w[i].T  (k, o)
    wgcn = np.concatenate([gcn_w[i].T for i in range(cfg.depth)], axis=1).astype(
        np.float32
    )
    gb = np.stack(
        sum([[gamma[i], beta[i]] for i in range(cfg.depth)], []), axis=1
    ).astype(np.float32)  # [D, 2*depth]: columns g0,b0,g1,b1

    in_maps = []
    for r in range(cfg.ncores):
        xs = x[r * npl : (r + 1) * npl]
        xT = np.zeros((D, npad), dtype=np.float32)
        xT[:, :npl] = xs.T
        in_maps.append(
            {
                "xT": np.ascontiguousarray(xT),
                "wlin": np.ascontiguousarray(wlin),
                "wgcn": np.ascontiguousarray(wgcn),
                "gb": np.ascontiguousarray(gb),
                "gidx": np.ascontiguousarray(idx_all[r]),
                "mt": np.ascontiguousarray(mt_all[r]),
            }
        )
    return in_maps


# ---------------------------------------------------------------- bass program
def _build_program(cfg, K):
    from concourse import bacc, mybir, tile

    f32 = mybir.dt.float32
    bf16 = mybir.dt.bfloat16
    i16 = mybir.dt.int16
    npl, npad, nt, nb = cfg.np_local, cfg.npad, cfg.nt, cfg.nb
    rg = [list(range(cfg.ncores))]
    inv_n = 1.0 / cfg.n_global
    skips = set(os.environ.get("KERNEL_SKIP", "").split(","))
    maxcalls = int(os.environ.get("KERNEL_MAXCALLS", "999999"))
    ncalls_done = 0

    chunk_meta, call_meta, cidx, batches = _layout(cfg, K)
    nchunks = len(chunk_meta)
    ktot = sum(K)
    bseq = [(b, j) for b in range(nb) if K[b] > 0 for j in range(K[b])]
    gcols = cfg.tb * ktot * 128  # max chunk-columns per batch buffer

    nc = bacc.Bacc(
        "TRN2", target_bir_lowering=False, debug=False, num_devices=cfg.ncores
    )

    xT = nc.dram_tensor("xT", [D, npad], f32, kind="ExternalInput")
    wlin_d = nc.dram_tensor("wlin", [D, cfg.depth * D], f32, kind="ExternalInput")
    wgcn_d = nc.dram_tensor("wgcn", [D, cfg.depth * D], f32, kind="ExternalInput")
    gb_d = nc.dram_tensor("gb", [D, 2 * cfg.depth], f32, kind="ExternalInput")
    gidx_d = nc.dram_tensor("gidx", [128, nchunks * 8], i16, kind="ExternalInput")
    mt_d = nc.dram_tensor("mt", [128, nchunks * 128], bf16, kind="ExternalInput")
    outT_d = nc.dram_tensor("outT", [D, npl], f32, kind="ExternalOutput")

    with tile.TileContext(nc) as tc:
        with (
            tc.tile_pool(name="big", bufs=1) as big,
            tc.tile_pool(name="gpool", bufs=2) as gpool,
            tc.tile_pool(name="mpool", bufs=2) as mpool,
            tc.tile_pool(name="ipool", bufs=2) as ipool,
            tc.tile_pool(name="cpool", bufs=4) as cpool,
            tc.tile_pool(name="small", bufs=1) as small,
            tc.tile_pool(name="stats", bufs=2) as stats_pool,
            tc.tile_pool(name="psA", bufs=4, space="PSUM") as psA,
            tc.tile_pool(name="psH", bufs=2, space="PSUM") as psH,
            tc.tile_pool(name="dram", bufs=1, space="DRAM") as dpool,
        ):
            hA = big.tile([128, npad], f32)
            hB = big.tile([128, npad], f32)
            consts = small.tile([128, 2], f32)  # col0 = 0.0, col1 = eps
            nc.vector.memset(consts[:, 0:1], 0.0)
            nc.vector.memset(consts[:, 1:2], float(cfg.eps))
            wlin = small.tile([128, cfg.depth * D], f32)
            wgcn = small.tile([128, cfg.depth * D], f32)
            gb = small.tile([128, 2 * cfg.depth], f32)

            nc.sync.dma_start(out=hA[:, :], in_=xT[:, :])
            nc.sync.dma_start(out=wlin[:, :], in_=wlin_d[:, :])
            nc.sync.dma_start(out=wgcn[:, :], in_=wgcn_d[:, :])
            nc.sync.dma_start(out=gb[:, :], in_=gb_d[:, :])

            cur, nxt = hA, hB
            for layer in range(cfg.depth):
                # ---- phase A: hg = h @ gcn_w.T (node-major, bf16) -> cc_in
                cc_in = dpool.tile([npl, D], bf16, name=f"cc_in_{layer}")
                cc_out = dpool.tile(
                    [cfg.n_global, D], bf16, addr_space="Shared", name=f"cc_out_{layer}"
                )
                wg = wgcn[:, layer * D : (layer + 1) * D]
                GA = 8  # dest tiles per store group (one wide DMA each)
                for t0a in range(0, nt, GA):
                    t1a = min(nt, t0a + GA)
                    strip = cpool.tile(
                        [128, GA * 128], bf16, name="sb_hg", tag="sb_hg"
                    )
                    for t in range(t0a, t1a):
                        ps = psA.tile([128, 128], f32, name="ps_hg", tag="ps_hg")
                        nc.tensor.matmul(
                            ps[:, :],
                            lhsT=cur[:, t * 128 : (t + 1) * 128],
                            rhs=wg,
                            start=True,
                            stop=True,
                        )
                        nc.vector.tensor_copy(
                            strip[:, (t - t0a) * 128 : (t - t0a + 1) * 128], ps[:, :]
                        )
                    if "astore" in skips:
                        continue
                    n0 = t0a * 128
                    nfull = (min(npl, t1a * 128) - n0) // 128  # whole 128-row tiles
                    if nfull > 0:
                        nc.sync.dma_start(
                            out=cc_in[n0 : n0 + nfull * 128, :].rearrange(
                                "(t p) e -> p t e", p=128
                            ),
                            in_=strip[:, : nfull * 128].rearrange(
                                "p (t e) -> p t e", e=128
                            ),
                        )
                    rem = min(npl, t1a * 128) - (n0 + nfull * 128)
                    if rem > 0:
                        nc.sync.dma_start(
                            out=cc_in[n0 + nfull * 128 :, :],
                            in_=strip[:rem, nfull * 128 : (nfull + 1) * 128],
                        )

                # ---- phase B: AllGather hg
                if "ag" not in skips:
                    nc.gpsimd.collective_compute(
                        "AllGather",
                        mybir.AluOpType.bypass,
                        replica_groups=rg,
                        ins=[cc_in[:, :].opt()],
                        outs=[cc_out[:, :].opt()],
                    )

                # ---- phase C: hl = h @ lin_w.T  (transposed layout, into nxt)
                wl = wlin[:, layer * D : (layer + 1) * D]
                if "hl" in skips:
                    nc.vector.memset(nxt[:, :], 0.0)
                c0 = 0
                while c0 < npad and "hl" not in skips:
                    cw = min(512, npad - c0)
                    ps = psH.tile([128, 512], f32, name="ps_hl", tag="ps_hl")
                    nc.tensor.matmul(
                        ps[:, :cw],
                        lhsT=wl,
                        rhs=cur[:, c0 : c0 + cw],
                        start=True,
                        stop=True,
                    )
                    nc.vector.tensor_copy(nxt[:, c0 : c0 + cw], ps[:, :cw])
                    c0 += cw

                # ---- phase D: bucketed dma_gather + one-hot matmul scatter-add
                for t0, t1, bc0, bc1 in batches:
                    nch_b = bc1 - bc0
                    g = gpool.tile([128, gcols], bf16, name="gbuf", tag="gbuf")
                    m = mpool.tile([128, gcols], bf16, name="mbuf", tag="mbuf")
                    wb = ipool.tile([128, cfg.tb * ktot * 8], i16, name="wb", tag="wb")
                    nc.sync.dma_start(
                        out=wb[:, : nch_b * 8], in_=gidx_d[:, bc0 * 8 : bc1 * 8]
                    )
                    if "mt" not in skips:
                        nc.sync.dma_start(
                            out=m[:, : nch_b * 128],
                            in_=mt_d[:, bc0 * 128 : bc1 * 128],
                        )
                    if "gather" not in skips:
                        for bi2, b, c0g, ncall in call_meta:
                            if not (bc0 <= c0g < bc1):
                                continue
                            if ncalls_done >= maxcalls:
                                continue
                            ncalls_done += 1
                            cl0 = c0g - bc0
                            out3 = g[
                                :, cl0 * 128 : (cl0 + ncall) * 128
                            ].rearrange("p (c e) -> p c e", e=128)
                            nc.gpsimd.dma_gather(
                                out3,
                                cc_out[cfg.bases[b] :, :],
                                wb[:, cl0 * 8 : (cl0 + ncall) * 8],
                                ncall * 128,
                                ncall * 128,
                                128,
                                single_packet=False,
                            )
                    if "aggmm" in skips:
                        continue
                    for t in range(t0, t1):
                        ps = psA.tile([128, 128], f32, name="ps_agg", tag="ps_hg")
                        for si, (b, j) in enumerate(bseq):
                            cl = cidx[(t, b, j)] - bc0
                            nc.tensor.matmul(
                                ps[:, :],
                                lhsT=g[:, cl * 128 : (cl + 1) * 128],
                                rhs=m[:, cl * 128 : (cl + 1) * 128],
                                start=(si == 0),
                                stop=(si == len(bseq) - 1),
                            )
                        nc.vector.tensor_add(
                            nxt[:, t * 128 : (t + 1) * 128],
                            nxt[:, t * 128 : (t + 1) * 128],
                            ps[:, :],
                        )

                # ---- phase E: BatchNorm stats + AllReduce
                skip_bn = "bn" in skips
                st = stats_pool.tile([128, 2], f32, name=f"st_{layer}")
                if not skip_bn:
                    nc.vector.reduce_sum(
                        out=st[:, 0:1], in_=nxt[:, :npl], axis=mybir.AxisListType.X
                    )
                if not skip_bn:
                    nc.scalar.activation(
                        out=cur[:, :npl],
                        in_=nxt[:, :npl],
                        func=mybir.ActivationFunctionType.Square,
                        bias=consts[:, 0:1],
                        accum_out=st[:, 1:2],
                    )
                if skip_bn:
                    nc.vector.memset(st[:, :], 1.0)
                bn_in = dpool.tile([128, 2], f32, name=f"bn_in_{layer}")
                bn_out = dpool.tile(
                    [128, 2], f32, addr_space="Shared", name=f"bn_out_{layer}"
                )
                nc.sync.dma_start(out=bn_in[:, :], in_=st[:, :])
                if "ar" not in skips:
                    nc.gpsimd.collective_compute(
                        "AllReduce",
                        mybir.AluOpType.add,
                        replica_groups=rg,
                        ins=[bn_in[:, :].opt()],
                        outs=[bn_out[:, :].opt()],
                    )
                gst = stats_pool.tile([128, 2], f32, name=f"gst_{layer}")
                nc.sync.dma_start(out=gst[:, :], in_=bn_out[:, :])

                # scale = gamma * rsqrt(var+eps); bias = beta - mean*scale
                mu = stats_pool.tile([128, 1], f32, name=f"mu_{layer}")
                vr = stats_pool.tile([128, 1], f32, name=f"vr_{layer}")
                sc = stats_pool.tile([128, 1], f32, name=f"sc_{layer}")
                bi = stats_pool.tile([128, 1], f32, name=f"bi_{layer}")
                nc.scalar.mul(mu[:, :], gst[:, 0:1], inv_n)  # mean
                nc.vector.tensor_scalar(
                    out=vr[:, :],
                    in0=gst[:, 1:2],
                    scalar1=inv_n,
                    scalar2=None,
                    op0=mybir.AluOpType.mult,
                )
                mu2 = stats_pool.tile([128, 1], f32, name=f"mu2_{layer}")
                nc.scalar.activation(
                    out=mu2[:, :],
                    in_=mu[:, :],
                    func=mybir.ActivationFunctionType.Square,
                    bias=consts[:, 0:1],
                )
                nc.vector.tensor_sub(vr[:, :], vr[:, :], mu2[:, :])
                nc.scalar.activation(
                    out=vr[:, :],
                    in_=vr[:, :],
                    func=mybir.ActivationFunctionType.Sqrt,
                    bias=consts[:, 1:2],
                )
                nc.vector.reciprocal(vr[:, :], vr[:, :])  # rstd
                nc.vector.tensor_mul(sc[:, :], vr[:, :], gb[:, 2 * layer : 2 * layer + 1])
                nc.vector.tensor_mul(bi[:, :], mu[:, :], sc[:, :])
                nc.vector.tensor_sub(bi[:, :], gb[:, 2 * layer + 1 : 2 * layer + 2], bi[:, :])

                # ---- phase F: apply BN (+ReLU except last layer), into cur
                if "apply" in skips:
                    if layer == cfg.depth - 1:
                        nc.sync.dma_start(out=outT_d[:, :], in_=nxt[:, :npl])
                    continue
                func = (
                    mybir.ActivationFunctionType.Relu
                    if layer != cfg.depth - 1
                    else mybir.ActivationFunctionType.Identity
                )
                nc.scalar.activation(
                    out=cur[:, :],
                    in_=nxt[:, :],
                    func=func,
                    bias=bi[:, :],
                    scale=sc[:, :],
                )
                # cur now holds the layer output (transposed); nxt is free
                if layer == cfg.depth - 1:
                    nc.sync.dma_start(out=outT_d[:, :], in_=cur[:, :npl])

    nc.compile()
    return nc


# ---------------------------------------------------------------- entry points
def _make_runner(cfg, nc, in_maps):
    """Build a repeat-callable PJRT runner with device-resident inputs (no
    donation) for wall-clock timing. Returns (run_once, fetch_results)."""
    import jax
    from jax.experimental.shard_map import shard_map
    from jax.sharding import Mesh, NamedSharding, PartitionSpec

    from concourse import bass2jax, mybir

    bass2jax.install_neuronx_cc_hook()

    partition_name = nc.partition_id_tensor.name if nc.partition_id_tensor else None
    in_names, out_names, out_avals, zero_outs = [], [], [], []
    for alloc in nc.m.functions[0].allocations:
        if not isinstance(alloc, mybir.MemoryLocationSet):
            continue
        name = alloc.memorylocations[0].name
        if alloc.kind == "ExternalInput":
            if name != partition_name:
                in_names.append(name)
        elif alloc.kind == "ExternalOutput":
            out_names.append(name)
            shape = tuple(alloc.tensor_shape)
            dtype = mybir.dt.np(alloc.dtype)
            out_avals.append(jax.core.ShapedArray(shape, dtype))
            zero_outs.append(np.zeros(shape, dtype))
    n_params = len(in_names)
    all_in_names = list(in_names) + list(out_names)
    if partition_name is not None:
        all_in_names.append(partition_name)

    def _body(*args):
        operands = list(args)
        if partition_name is not None:
            operands.append(bass2jax.partition_id_tensor())
        outs = bass2jax._bass_exec_p.bind(
            *operands,
            out_avals=tuple(out_avals),
            in_names=tuple(all_in_names),
            out_names=tuple(out_names),
            lowering_input_output_aliases=(),
            sim_require_finite=True,
            sim_require_nnan=True,
            nc=nc,
        )
        return tuple(outs)

    n = cfg.ncores
    devices = jax.devices()[:n]
    mesh = Mesh(np.asarray(devices), ("core",))
    n_outs = len(out_names)
    in_specs = (PartitionSpec("core"),) * (n_params + n_outs)
    out_specs = (PartitionSpec("core"),) * n_outs
    sharded = jax.jit(
        shard_map(
            _body, mesh=mesh, in_specs=in_specs, out_specs=out_specs, check_rep=False
        ),
        keep_unused=True,
    )
    shd = NamedSharding(mesh, PartitionSpec("core"))
    concat_in = [
        jax.device_put(
            np.concatenate([np.asarray(in_maps[c][k]) for c in range(n)], axis=0), shd
        )
        for k in in_names
    ]
    concat_zeros = [
        jax.device_put(np.zeros((n * z.shape[0], *z.shape[1:]), z.dtype), shd)
        for z in zero_outs
    ]

    def run_once():
        outs = sharded(*concat_in, *concat_zeros)
        jax.block_until_ready(outs)
        return outs

    def fetch(outs):
        return [
            {
                k: np.asarray(outs[i]).reshape(n, *out_avals[i].shape)[c]
                for i, k in enumerate(out_names)
            }
            for c in range(n)
        ]

    return run_once, fetch


def _assemble(cfg, results):
    out = np.empty((cfg.n_global, D), dtype=np.float32)
    npl = cfg.np_local
    for r in range(cfg.ncores):
        out[r * npl : (r + 1) * npl] = results[r]["outT"].T
    return out


def kernel(x, edge_index, edge_weight, lin_w, gcn_w, gamma, beta):
    global LAST_RUNNER
    cfg = _Cfg(N_GLOBAL, NCORES)
    x = np.asarray(x)
    assert x.shape == (cfg.n_global, D)
    K, idx_all, mt_all = _prep_graph(cfg, np.asarray(edge_index), np.asarray(edge_weight))
    in_maps = _prep_inputs(
        cfg, K, idx_all, mt_all, x, np.asarray(lin_w), np.asarray(gcn_w),
        np.asarray(gamma), np.asarray(beta),
    )
    nc = _build_program(cfg, K)
    run_once, fetch = _make_runner(cfg, nc, in_maps)
    LAST_RUNNER = (run_once, fetch)
    results = fetch(run_once())
    return _assemble(cfg, results)



# revision 35
# speedup vs baseline: 3.7180x; 3.7180x over previous
"""DiGCN (2-layer GCNConv + parallel Linear + BatchNorm1d + ReLU) on 8 trn2 NeuronCores.

Strategy (matches the problem's sharding hint):
  - Shard nodes contiguously across 8 cores (12500 nodes/core), replicate the
    small [D,D] weights, partition edges by destination-node owner.
  - Per layer: each core computes hg = h_local @ gcn_w.T (bf16), AllGathers hg
    so every core holds the full [N,D] table; gathers its edges' source rows
    via the GPSIMD dma_gather extended instruction; scatter-adds into its local
    destination nodes via one-hot matmuls on the TensorEngine (edges sorted by
    dest tile; a host-built [128e x 128d] one-hot*norm bf16 matrix per 128-edge
    chunk turns segment-sum into PSUM accumulation). BN stats via AllReduce.
  - dma_gather indices are int16, so the [N,D] table is addressed through
    range buckets of 32767 rows; slots are laid out bucket-major per batch so
    each (batch, bucket) is one packed gather call.
  - h is kept transposed in SBUF ([128 feat, nodes], f32) so BN reduction is a
    free-dim reduce and BN+ReLU fuse into one ScalarE activation pass.

kernel(**inputs) takes FULL inputs, returns the FULL [N,D] float32 output.
"""

import math
import os
import sys

import numpy as np

for _p in ("/opt/trn_rl_repo", "/root/.axon_site/_ro/trn_rl_repo"):
    if os.path.isdir(_p) and _p not in sys.path:
        sys.path.insert(0, _p)

# ---------------------------------------------------------------- configuration
N_GLOBAL = 100000
E_GLOBAL = 500000
D = 128
DEPTH = 2
EPS = 1e-5
NCORES = 8
IDXMAX = 32767  # dma_gather int16 index limit (rows per range bucket)

LAST_RUNNER = None  # (run_once, fetch) of the most recent kernel() call
LAST_NC = None  # Bacc program of the most recent kernel() call


class _Cfg:
    def __init__(self, n_global, ncores, depth=DEPTH, eps=EPS, idxmax=IDXMAX, tb=8):
        assert n_global % ncores == 0
        self.n_global = n_global
        self.ncores = ncores
        self.depth = depth
        self.eps = eps
        self.idxmax = idxmax
        self.tb = tb                                # dest tiles per gather batch
        self.np_local = n_global // ncores          # real nodes per core
        self.nt = math.ceil(self.np_local / 128)    # dest tiles per core
        self.npad = self.nt * 128                   # padded nodes per core
        # gather range buckets: 4 equal buckets of n/4 (< 32767) so buckets
        # 0,1 lie in the first AllGather half and 2,3 in the second
        self.bsize = n_global // 4
        assert self.bsize <= idxmax and n_global % 4 == 0
        self.nb = 4
        self.bases = [b * self.bsize for b in range(self.nb)]


def _layout(cfg, K):
    """Chunk/call enumeration shared by host prep and the bass builder.

    Returns (chunk_meta, call_meta, cidx, batches):
      chunk_meta[c] = (bucket, tile, j)
      call_meta = list of (batch_idx, bucket, c0, nchunks_in_call)
      cidx[(t, b, j)] = global chunk index
      batches = list of (t0, t1, bc0, bc1)  (tile range, chunk range)
    """
    chunk_meta, call_meta, cidx, batches = [], [], {}, []
    nbatch = math.ceil(cfg.nt / cfg.tb)
    for bi in range(nbatch):
        t0, t1 = bi * cfg.tb, min(cfg.nt, bi * cfg.tb + cfg.tb)
        bc0 = len(chunk_meta)
        for b in range(cfg.nb):
            if K[b] == 0:
                continue
            c0 = len(chunk_meta)
            for t in range(t0, t1):
                for j in range(K[b]):
                    cidx[(t, b, j)] = len(chunk_meta)
                    chunk_meta.append((b, t, j))
            call_meta.append((bi, b, c0, len(chunk_meta) - c0))
        batches.append((t0, t1, bc0, len(chunk_meta)))
    return chunk_meta, call_meta, cidx, batches


def _prep_graph_packed(cfg, edge_index, edge_weight):
    """Packed-slot layout: per (batch, bucket) gather call, each core's edges
    occupy slots [0, cnt) contiguously (sorted by dest tile); trailing slots
    are -1 so the Q7 gather trims them per-core (no desc-gen, no DMA).
    Chunks cross tile boundaries; the aggregation matmul schedule is the
    per-(chunk, tile) instance union over cores.

    Returns (meta, idx_all, mt_all)."""
    import ml_dtypes

    row = np.asarray(edge_index[0], dtype=np.int64)
    col = np.asarray(edge_index[1], dtype=np.int64)
    w = np.asarray(edge_weight, dtype=np.float32)
    n, nb, nt, npl, tb = cfg.n_global, cfg.nb, cfg.nt, cfg.np_local, cfg.tb
    nbatch = math.ceil(nt / tb)
    nc_ = cfg.ncores

    deg = np.bincount(col, weights=w.astype(np.float64), minlength=n).astype(
        np.float32
    )
    dinv = np.where(deg > 0, 1.0 / np.sqrt(np.where(deg > 0, deg, 1.0)), 0.0).astype(
        np.float32
    )
    norm = (dinv[row] * w * dinv[col]).astype(np.float32)

    core = col // npl
    lc = col % npl
    tile = lc // 128
    d_in_tile = lc % 128
    bi_e = tile // tb
    # gather-table position of source row: half-core-major so the AllGather
    # can be split in two (halves of every core's hg) and bucket-0 gathers
    # start after the first half lands
    half = npl // 2
    c_src = row // npl
    o_src = row % npl
    pos = np.where(
        o_src < half,
        c_src * half + o_src,
        n // 2 + c_src * half + (o_src - half),
    )
    bucket = pos // cfg.bsize

    # sort by (core, batch, bucket, tile); rank within (core, batch, bucket)
    key_full = ((core * nbatch + bi_e) * nb + bucket) * nt + tile
    order = np.argsort(key_full, kind="stable")
    grp = (core * nbatch + bi_e) * nb + bucket
    grp_s = grp[order]
    counts_g = np.bincount(grp, minlength=nc_ * nbatch * nb)
    starts_g = np.concatenate([[0], np.cumsum(counts_g)[:-1]])
    rank = np.arange(row.shape[0], dtype=np.int64) - starts_g[grp_s]

    cnt3 = counts_g.reshape(nc_, nbatch, nb)
    max_cnt = cnt3.max(axis=0)  # [nbatch, nb] valid count per call (all cores)
    K2 = np.ceil(max_cnt / 128).astype(np.int64)  # [nbatch, nb]

    # per-(core, batch, bucket, tile) start/end ranks for instance ranges
    key_t = ((core * nbatch + bi_e) * nb + bucket) * nt + tile
    counts_t = np.bincount(key_t, minlength=nc_ * nbatch * nb * nt).reshape(
        nc_, nbatch, nb, nt
    )
    start_t = np.cumsum(counts_t, axis=3) - counts_t  # exclusive prefix
    end_t = start_t + counts_t

    # chunk layout + instance enumeration
    batches = []
    nchunks = 0
    ninst = 0
    bc0_arr = np.zeros((nbatch, nb), dtype=np.int64)
    iid_map = {}
    for bi in range(nbatch):
        t0, t1 = bi * tb, min(nt, bi * tb + tb)
        bc0 = nchunks
        calls = []
        for b in range(nb):
            k = int(K2[bi, b])
            bc0_arr[bi, b] = nchunks
            if k > 0:
                calls.append((b, nchunks, k, int(max_cnt[bi, b])))
                nchunks += k
        i0 = ninst
        tiles = []
        for t in range(t0, t1):
            insts = []
            for b in range(nb):
                has = counts_t[:, bi, b, t] > 0
                if not has.any():
                    continue
                clo = int(
                    (start_t[has, bi, b, t] // 128).min() + bc0_arr[bi, b]
                )
                chi = int(
                    ((end_t[has, bi, b, t] - 1) // 128).max() + bc0_arr[bi, b]
                )
                for c in range(clo, chi + 1):
                    iid_map[(bi, b, t, c)] = ninst
                    insts.append((ninst, c))
                    ninst += 1
            tiles.append((t, insts))
        batches.append(
            dict(t0=t0, t1=t1, bc0=bc0, bc1=nchunks, i0=i0, i1=ninst,
                 calls=calls, tiles=tiles)
        )

    # per-edge slot -> chunk/partition; instance id
    bi_s = bi_e[order]
    b_s = bucket[order]
    core_s = core[order]
    t_s = tile[order]
    c_s = bc0_arr[bi_s, b_s] + rank // 128
    p_s = rank % 128
    iid_s = np.fromiter(
        (iid_map[(bi, b, t, c)] for bi, b, t, c in zip(bi_s, b_s, t_s, c_s)),
        dtype=np.int64,
        count=len(order),
    )

    base_arr = np.asarray(cfg.bases, dtype=np.int64)
    val_s = (pos[order] - base_arr[b_s]).astype(np.int16)
    assert (val_s >= 0).all()

    idx_all = np.full((nc_, 128, nchunks * 8), -1, dtype=np.int16)
    idx_all[core_s, p_s % 16, c_s * 8 + p_s // 16] = val_s
    # pad every core's call region to the same valid count (max over cores)
    # with dummy index 0: num_idxs_reg is static in the shared program, and
    # the decode-side ring reservation must equal what the Q7 pushes after
    # the trailing -1 trim.
    for ci in range(nc_):
        for bi in range(nbatch):
            for b in range(nb):
                cnt = int(cnt3[ci, bi, b])
                mc = int(max_cnt[bi, b])
                if cnt >= mc:
                    continue
                r = np.arange(cnt, mc, dtype=np.int64)
                cpad = bc0_arr[bi, b] + r // 128
                ppad = r % 128
                idx_all[ci, ppad % 16, cpad * 8 + ppad // 16] = 0
    for s in range(1, 8):
        idx_all[:, 16 * s : 16 * (s + 1), :] = idx_all[:, 0:16, :]

    mt_all = np.zeros((nc_, 128, ninst * 128), dtype=ml_dtypes.bfloat16)
    mt_all[core_s, p_s, iid_s * 128 + d_in_tile[order]] = norm[order].astype(
        ml_dtypes.bfloat16
    )

    meta = dict(nchunks=nchunks, ninst=ninst, batches=batches)
    return meta, idx_all, mt_all


# ---------------------------------------------------------------- host-side prep
def _prep_graph(cfg, edge_index, edge_weight):
    """Partition edges by destination owner/tile/src-bucket; build int16 gather
    indices and the per-chunk one-hot*norm matrices (bf16)."""
    import ml_dtypes

    row = np.asarray(edge_index[0], dtype=np.int64)
    col = np.asarray(edge_index[1], dtype=np.int64)
    w = np.asarray(edge_weight, dtype=np.float32)
    n, nb, nt, npl = cfg.n_global, cfg.nb, cfg.nt, cfg.np_local

    deg = np.bincount(col, weights=w.astype(np.float64), minlength=n).astype(np.float32)
    dinv = np.where(deg > 0, 1.0 / np.sqrt(np.where(deg > 0, deg, 1.0)), 0.0).astype(
        np.float32
    )
    norm = (dinv[row] * w * dinv[col]).astype(np.float32)

    core = col // npl
    lc = col % npl
    tile = lc // 128
    d_in_tile = lc % 128
    bucket = np.minimum(row // cfg.idxmax, nb - 1)

    key = (core * nt + tile) * nb + bucket
    order = np.argsort(key, kind="stable")
    key_s = key[order]
    counts = np.bincount(key, minlength=cfg.ncores * nt * nb)
    starts = np.concatenate([[0], np.cumsum(counts)[:-1]])
    rank = np.arange(row.shape[0], dtype=np.int64) - starts[key_s]

    cnt3 = counts.reshape(cfg.ncores, nt, nb)
    K = [int(math.ceil(cnt3[:, :, b].max() / 128)) for b in range(nb)]

    chunk_meta, call_meta, cidx, batches = _layout(cfg, K)
    nchunks = len(chunk_meta)

    # per-edge slot: chunk c = cidx[(tile, bucket, rank//128)], partition rank%128
    cidx_arr = np.full((nt, nb, max(K)), -1, dtype=np.int64)
    for (t, b, j), c in cidx.items():
        cidx_arr[t, b, j] = c
    t_s = (key_s // nb) % nt
    b_s = key_s % nb
    core_s = key_s // (nb * nt)
    j_s = rank // 128
    p_s = rank % 128
    c_s = cidx_arr[t_s, b_s, j_s]
    assert (c_s >= 0).all()

    base_arr = np.asarray(cfg.bases, dtype=np.int64)
    val_s = (row[order] - base_arr[b_s]).astype(np.int16)
    assert (val_s >= 0).all()

    # int16 gather indices: element i of a call lives at [i%16, chunk*8 + p//16]
    idx_all = np.zeros((cfg.ncores, 128, nchunks * 8), dtype=np.int16)
    idx_all[core_s, p_s % 16, c_s * 8 + p_s // 16] = val_s
    # replicate across all 8 Q7 cpu slices so any swdge queue (cpu pair
    # 2q,2q+1 reads partitions 32q..32q+31) sees the indices
    for s in range(1, 8):
        idx_all[:, 16 * s : 16 * (s + 1), :] = idx_all[:, 0:16, :]

    mt_all = np.zeros((cfg.ncores, 128, nchunks * 128), dtype=ml_dtypes.bfloat16)
    mt_all[core_s, p_s, c_s * 128 + d_in_tile[order]] = norm[order].astype(
        ml_dtypes.bfloat16
    )
    return K, idx_all, mt_all


def _prep_inputs(cfg, K, idx_all, mt_all, x, lin_w, gcn_w, gamma, beta):
    x = np.asarray(x, dtype=np.float32)
    npl, npad = cfg.np_local, cfg.npad
    wlin = np.concatenate([lin_w[i].T for i in range(cfg.depth)], axis=1).astype(
        np.float32
    )  # [D, depth*D], column block i = lin_w[i].T  (k, o)
    wgcn = np.concatenate([gcn_w[i].T for i in range(cfg.depth)], axis=1).astype(
        np.float32
    )
    gb = np.stack(
        sum([[gamma[i], beta[i]] for i in range(cfg.depth)], []), axis=1
    ).astype(np.float32)  # [D, 2*depth]: columns g0,b0,g1,b1

    in_maps = []
    for r in range(cfg.ncores):
        xs = x[r * npl : (r + 1) * npl]
        xT = np.zeros((D, npad), dtype=np.float32)
        xT[:, :npl] = xs.T
        in_maps.append(
            {
                "xT": np.ascontiguousarray(xT),
                "wlin": np.ascontiguousarray(wlin),
                "wgcn": np.ascontiguousarray(wgcn),
                "gb": np.ascontiguousarray(gb),
                "gidx": np.ascontiguousarray(idx_all[r]),
                "mt": np.ascontiguousarray(mt_all[r]),
            }
        )
    return in_maps


# ---------------------------------------------------------------- bass program
def _build_program(cfg, meta):
    from concourse import bacc, mybir, tile

    f32 = mybir.dt.float32
    bf16 = mybir.dt.bfloat16
    i16 = mybir.dt.int16
    npl, npad, nt, nb = cfg.np_local, cfg.npad, cfg.nt, cfg.nb
    rg = [list(range(cfg.ncores))]
    inv_n = 1.0 / cfg.n_global
    skips = set(os.environ.get("KERNEL_SKIP", "").split(","))
    maxcalls = int(os.environ.get("KERNEL_MAXCALLS", "999999"))
    nq = int(os.environ.get("KERNEL_NSWDGE", "3"))
    ncalls_done = 0

    nchunks = meta["nchunks"]
    ninst = meta["ninst"]
    batches = meta["batches"]
    gcols = max(bat["bc1"] - bat["bc0"] for bat in batches) * 128
    micols = max(bat["i1"] - bat["i0"] for bat in batches) * 128
    wcols = max(bat["bc1"] - bat["bc0"] for bat in batches) * 8

    nc = bacc.Bacc(
        "TRN2",
        target_bir_lowering=False,
        debug=False,
        num_devices=cfg.ncores,
        num_swdge_queues=int(os.environ.get("KERNEL_NSWDGE", "3")),
    )

    xT = nc.dram_tensor("xT", [D, npad], f32, kind="ExternalInput")
    wlin_d = nc.dram_tensor("wlin", [D, cfg.depth * D], f32, kind="ExternalInput")
    wgcn_d = nc.dram_tensor("wgcn", [D, cfg.depth * D], f32, kind="ExternalInput")
    gb_d = nc.dram_tensor("gb", [D, 2 * cfg.depth], f32, kind="ExternalInput")
    gidx_d = nc.dram_tensor("gidx", [128, nchunks * 8], i16, kind="ExternalInput")
    mt_d = nc.dram_tensor("mt", [128, ninst * 128], bf16, kind="ExternalInput")
    outT_d = nc.dram_tensor("outT", [D, npl], f32, kind="ExternalOutput")

    with tile.TileContext(nc) as tc:
        with (
            tc.tile_pool(name="big", bufs=1) as big,
            tc.tile_pool(name="gpool", bufs=4) as gpool,
            tc.tile_pool(name="mpool", bufs=4) as mpool,
            tc.tile_pool(name="ipool", bufs=4) as ipool,
            tc.tile_pool(name="cpool", bufs=4) as cpool,
            tc.tile_pool(name="small", bufs=1) as small,
            tc.tile_pool(name="stats", bufs=2) as stats_pool,
            tc.tile_pool(name="psA", bufs=4, space="PSUM") as psA,
            tc.tile_pool(name="psH", bufs=2, space="PSUM") as psH,
            tc.tile_pool(name="dram", bufs=1, space="DRAM") as dpool,
        ):
            hA = big.tile([128, npad], f32)
            hB = big.tile([128, npad], f32)
            consts = small.tile([128, 2], f32)  # col0 = 0.0, col1 = eps
            nc.vector.memset(consts[:, 0:1], 0.0)
            nc.vector.memset(consts[:, 1:2], float(cfg.eps))
            wlin = small.tile([128, cfg.depth * D], f32)
            wgcn = small.tile([128, cfg.depth * D], f32)
            gb = small.tile([128, 2 * cfg.depth], f32)

            nc.sync.dma_start(out=hA[:, :], in_=xT[:, :])
            nc.sync.dma_start(out=wlin[:, :], in_=wlin_d[:, :])
            nc.sync.dma_start(out=wgcn[:, :], in_=wgcn_d[:, :])
            nc.sync.dma_start(out=gb[:, :], in_=gb_d[:, :])

            cur, nxt = hA, hB
            for layer in range(cfg.depth):
                # ---- phase A: hg = h @ gcn_w.T (node-major, bf16) -> cc_in
                cc_in = dpool.tile([npl, D], bf16, name=f"cc_in_{layer}")
                cc_outa = dpool.tile(
                    [cfg.n_global // 2, D], bf16, addr_space="Shared",
                    name=f"cc_outa_{layer}",
                )
                cc_outb = dpool.tile(
                    [cfg.n_global // 2, D], bf16, addr_space="Shared",
                    name=f"cc_outb_{layer}",
                )
                wg = wgcn[:, layer * D : (layer + 1) * D]
                GA = 8  # dest tiles per store group (one wide DMA each)
                for t0a in range(0, nt, GA):
                    t1a = min(nt, t0a + GA)
                    strip = cpool.tile(
                        [128, GA * 128], bf16, name="sb_hg", tag="sb_hg"
                    )
                    for t in range(t0a, t1a):
                        ps = psA.tile([128, 128], f32, name="ps_hg", tag="ps_hg")
                        nc.tensor.matmul(
                            ps[:, :],
                            lhsT=cur[:, t * 128 : (t + 1) * 128],
                            rhs=wg,
                            start=True,
                            stop=True,
                        )
                        nc.vector.tensor_copy(
                            strip[:, (t - t0a) * 128 : (t - t0a + 1) * 128], ps[:, :]
                        )
                    if "astore" in skips:
                        continue
                    n0 = t0a * 128
                    nfull = (min(npl, t1a * 128) - n0) // 128  # whole 128-row tiles
                    if nfull > 0:
                        nc.sync.dma_start(
                            out=cc_in[n0 : n0 + nfull * 128, :].rearrange(
                                "(t p) e -> p t e", p=128
                            ),
                            in_=strip[:, : nfull * 128].rearrange(
                                "p (t e) -> p t e", e=128
                            ),
                        )
                    rem = min(npl, t1a * 128) - (n0 + nfull * 128)
                    if rem > 0:
                        nc.sync.dma_start(
                            out=cc_in[n0 + nfull * 128 :, :],
                            in_=strip[:rem, nfull * 128 : (nfull + 1) * 128],
                        )

                # ---- phase B: AllGather hg, split in two halves so the
                # first half's buckets can be gathered while the second
                # half is still in flight (table is half-core-major)
                half = npl // 2
                nhalf = cfg.n_global // 2
                if "ag" not in skips:
                    nc.gpsimd.collective_compute(
                        "AllGather",
                        mybir.AluOpType.bypass,
                        replica_groups=rg,
                        ins=[cc_in[0:half, :].opt()],
                        outs=[cc_outa[:, :].opt()],
                    )
                    nc.gpsimd.collective_compute(
                        "AllGather",
                        mybir.AluOpType.bypass,
                        replica_groups=rg,
                        ins=[cc_in[half:, :].opt()],
                        outs=[cc_outb[:, :].opt()],
                    )

                # ---- phase C: hl = h @ lin_w.T  (transposed layout, into nxt)
                wl = wlin[:, layer * D : (layer + 1) * D]
                if "hl" in skips:
                    nc.vector.memset(nxt[:, :], 0.0)
                c0 = 0
                while c0 < npad and "hl" not in skips:
                    cw = min(512, npad - c0)
                    ps = psH.tile([128, 512], f32, name="ps_hl", tag="ps_hl")
                    nc.tensor.matmul(
                        ps[:, :cw],
                        lhsT=wl,
                        rhs=cur[:, c0 : c0 + cw],
                        start=True,
                        stop=True,
                    )
                    nc.vector.tensor_copy(nxt[:, c0 : c0 + cw], ps[:, :cw])
                    c0 += cw

                # ---- phase D: packed dma_gather + per-instance matmul scatter-add
                qi = 0
                for bidx, bat in enumerate(batches):
                    bc0, bc1 = bat["bc0"], bat["bc1"]
                    i0 = bat["i0"]
                    nch_b = bc1 - bc0
                    ni_b = bat["i1"] - i0
                    g = gpool.tile([128, gcols], bf16, name="gbuf", tag="gbuf")
                    m = mpool.tile([128, micols], bf16, name="mbuf", tag="mbuf")
                    wb = ipool.tile([128, wcols], i16, name="wb", tag="wb")
                    nc.sync.dma_start(
                        out=wb[:, : nch_b * 8], in_=gidx_d[:, bc0 * 8 : bc1 * 8]
                    )
                    if "mt" not in skips:
                        nc.sync.dma_start(
                            out=m[:, : ni_b * 128],
                            in_=mt_d[:, i0 * 128 : bat["i1"] * 128],
                        )
                    if layer == 0 and bidx < 4:
                        # first use of each rotating g buffer: clear so trimmed
                        # (never-gathered) slots hold finite values, not
                        # uninitialized SBUF (0 * NaN = NaN in the matmul)
                        nc.vector.memset(g[:, :], 0.0)
                    if "gather" not in skips:
                        for b, c0g, ncall, nvalid in bat["calls"]:
                            if ncalls_done >= maxcalls:
                                continue
                            ncalls_done += 1
                            cl0 = c0g - bc0
                            out3 = g[
                                :, cl0 * 128 : (cl0 + ncall) * 128
                            ].rearrange("p (c e) -> p c e", e=128)
                            if b < 2:
                                src = cc_outa[
                                    cfg.bases[b] : cfg.bases[b] + cfg.bsize, :
                                ]
                            else:
                                src = cc_outb[
                                    cfg.bases[b] - nhalf
                                    : cfg.bases[b] - nhalf + cfg.bsize, :
                                ]
                            nc.gpsimd.dma_gather(
                                out3,
                                src,
                                wb[:, cl0 * 8 : (cl0 + ncall) * 8],
                                ncall * 128,
                                nvalid,
                                128,
                                single_packet=False,
                                queue_num=qi % nq,
                            )
                            qi += 1
                    if "aggmm" in skips:
                        continue
                    for t, insts in bat["tiles"]:
                        if not insts:
                            continue
                        ps = psA.tile([128, 128], f32, name="ps_agg", tag="ps_hg")
                        for si, (iid, c) in enumerate(insts):
                            cl = c - bc0
                            il = iid - i0
                            nc.tensor.matmul(
                                ps[:, :],
                                lhsT=g[:, cl * 128 : (cl + 1) * 128],
                                rhs=m[:, il * 128 : (il + 1) * 128],
                                start=(si == 0),
                                stop=(si == len(insts) - 1),
                            )
                        nc.vector.tensor_add(
                            nxt[:, t * 128 : (t + 1) * 128],
                            nxt[:, t * 128 : (t + 1) * 128],
                            ps[:, :],
                        )

                # ---- phase E: BatchNorm stats + AllReduce
                skip_bn = "bn" in skips
                st = stats_pool.tile([128, 2], f32, name=f"st_{layer}")
                if not skip_bn:
                    nc.vector.reduce_sum(
                        out=st[:, 0:1], in_=nxt[:, :npl], axis=mybir.AxisListType.X
                    )
                if not skip_bn:
                    nc.scalar.activation(
                        out=cur[:, :npl],
                        in_=nxt[:, :npl],
                        func=mybir.ActivationFunctionType.Square,
                        bias=consts[:, 0:1],
                        accum_out=st[:, 1:2],
                    )
                if skip_bn:
                    nc.vector.memset(st[:, :], 1.0)
                bn_in = dpool.tile([128, 2], f32, name=f"bn_in_{layer}")
                bn_out = dpool.tile(
                    [128, 2], f32, addr_space="Shared", name=f"bn_out_{layer}"
                )
                nc.sync.dma_start(out=bn_in[:, :], in_=st[:, :])
                if "ar" not in skips:
                    nc.gpsimd.collective_compute(
                        "AllReduce",
                        mybir.AluOpType.add,
                        replica_groups=rg,
                        ins=[bn_in[:, :].opt()],
                        outs=[bn_out[:, :].opt()],
                    )
                gst = stats_pool.tile([128, 2], f32, name=f"gst_{layer}")
                nc.sync.dma_start(out=gst[:, :], in_=bn_out[:, :])

                # scale = gamma * rsqrt(var+eps); bias = beta - mean*scale
                mu = stats_pool.tile([128, 1], f32, name=f"mu_{layer}")
                vr = stats_pool.tile([128, 1], f32, name=f"vr_{layer}")
                sc = stats_pool.tile([128, 1], f32, name=f"sc_{layer}")
                bi = stats_pool.tile([128, 1], f32, name=f"bi_{layer}")
                nc.scalar.mul(mu[:, :], gst[:, 0:1], inv_n)  # mean
                nc.vector.tensor_scalar(
                    out=vr[:, :],
                    in0=gst[:, 1:2],
                    scalar1=inv_n,
                    scalar2=None,
                    op0=mybir.AluOpType.mult,
                )
                mu2 = stats_pool.tile([128, 1], f32, name=f"mu2_{layer}")
                nc.scalar.activation(
                    out=mu2[:, :],
                    in_=mu[:, :],
                    func=mybir.ActivationFunctionType.Square,
                    bias=consts[:, 0:1],
                )
                nc.vector.tensor_sub(vr[:, :], vr[:, :], mu2[:, :])
                nc.scalar.activation(
                    out=vr[:, :],
                    in_=vr[:, :],
                    func=mybir.ActivationFunctionType.Sqrt,
                    bias=consts[:, 1:2],
                )
                nc.vector.reciprocal(vr[:, :], vr[:, :])  # rstd
                nc.vector.tensor_mul(sc[:, :], vr[:, :], gb[:, 2 * layer : 2 * layer + 1])
                nc.vector.tensor_mul(bi[:, :], mu[:, :], sc[:, :])
                nc.vector.tensor_sub(bi[:, :], gb[:, 2 * layer + 1 : 2 * layer + 2], bi[:, :])

                # ---- phase F: apply BN (+ReLU except last layer), into cur
                if "apply" in skips:
                    if layer == cfg.depth - 1:
                        nc.sync.dma_start(out=outT_d[:, :], in_=nxt[:, :npl])
                    continue
                func = (
                    mybir.ActivationFunctionType.Relu
                    if layer != cfg.depth - 1
                    else mybir.ActivationFunctionType.Identity
                )
                nc.scalar.activation(
                    out=cur[:, :],
                    in_=nxt[:, :],
                    func=func,
                    bias=bi[:, :],
                    scale=sc[:, :],
                )
                # cur now holds the layer output (transposed); nxt is free
                if layer == cfg.depth - 1:
                    nc.sync.dma_start(out=outT_d[:, :], in_=cur[:, :npl])

    nc.compile()
    return nc


# ---------------------------------------------------------------- entry points
def _make_runner(cfg, nc, in_maps):
    return _make_runner_n(nc, in_maps, cfg.ncores)


def _make_runner_n(nc, in_maps, ncores):
    """Build a repeat-callable PJRT runner with device-resident inputs (no
    donation) for wall-clock timing. Returns (run_once, fetch_results)."""
    import jax
    from jax.experimental.shard_map import shard_map
    from jax.sharding import Mesh, NamedSharding, PartitionSpec

    from concourse import bass2jax, mybir

    bass2jax.install_neuronx_cc_hook()

    partition_name = nc.partition_id_tensor.name if nc.partition_id_tensor else None
    in_names, out_names, out_avals, zero_outs = [], [], [], []
    for alloc in nc.m.functions[0].allocations:
        if not isinstance(alloc, mybir.MemoryLocationSet):
            continue
        name = alloc.memorylocations[0].name
        if alloc.kind == "ExternalInput":
            if name != partition_name:
                in_names.append(name)
        elif alloc.kind == "ExternalOutput":
            out_names.append(name)
            shape = tuple(alloc.tensor_shape)
            dtype = mybir.dt.np(alloc.dtype)
            out_avals.append(jax.core.ShapedArray(shape, dtype))
            zero_outs.append(np.zeros(shape, dtype))
    n_params = len(in_names)
    all_in_names = list(in_names) + list(out_names)
    if partition_name is not None:
        all_in_names.append(partition_name)

    def _body(*args):
        operands = list(args)
        if partition_name is not None:
            operands.append(bass2jax.partition_id_tensor())
        outs = bass2jax._bass_exec_p.bind(
            *operands,
            out_avals=tuple(out_avals),
            in_names=tuple(all_in_names),
            out_names=tuple(out_names),
            lowering_input_output_aliases=(),
            sim_require_finite=True,
            sim_require_nnan=True,
            nc=nc,
        )
        return tuple(outs)

    n = ncores
    devices = jax.devices()[:n]
    mesh = Mesh(np.asarray(devices), ("core",))
    n_outs = len(out_names)
    in_specs = (PartitionSpec("core"),) * (n_params + n_outs)
    out_specs = (PartitionSpec("core"),) * n_outs
    sharded = jax.jit(
        shard_map(
            _body, mesh=mesh, in_specs=in_specs, out_specs=out_specs, check_rep=False
        ),
        keep_unused=True,
    )
    shd = NamedSharding(mesh, PartitionSpec("core"))
    concat_in = [
        jax.device_put(
            np.concatenate([np.asarray(in_maps[c][k]) for c in range(n)], axis=0), shd
        )
        for k in in_names
    ]
    concat_zeros = [
        jax.device_put(np.zeros((n * z.shape[0], *z.shape[1:]), z.dtype), shd)
        for z in zero_outs
    ]

    def run_once():
        outs = sharded(*concat_in, *concat_zeros)
        jax.block_until_ready(outs)
        return outs

    def fetch(outs):
        return [
            {
                k: np.asarray(outs[i]).reshape(n, *out_avals[i].shape)[c]
                for i, k in enumerate(out_names)
            }
            for c in range(n)
        ]

    return run_once, fetch


def _assemble(cfg, results):
    out = np.empty((cfg.n_global, D), dtype=np.float32)
    npl = cfg.np_local
    for r in range(cfg.ncores):
        out[r * npl : (r + 1) * npl] = results[r]["outT"].T
    return out


def kernel(x, edge_index, edge_weight, lin_w, gcn_w, gamma, beta):
    global LAST_RUNNER, LAST_NC
    cfg = _Cfg(N_GLOBAL, NCORES, tb=int(os.environ.get("KERNEL_TB", "4")))
    x = np.asarray(x)
    assert x.shape == (cfg.n_global, D)
    meta, idx_all, mt_all = _prep_graph_packed(
        cfg, np.asarray(edge_index), np.asarray(edge_weight)
    )
    in_maps = _prep_inputs(
        cfg, meta, idx_all, mt_all, x, np.asarray(lin_w), np.asarray(gcn_w),
        np.asarray(gamma), np.asarray(beta),
    )
    nc = _build_program(cfg, meta)
    LAST_NC = nc
    run_once, fetch = _make_runner(cfg, nc, in_maps)
    LAST_RUNNER = (run_once, fetch)
    results = fetch(run_once())
    return _assemble(cfg, results)

